# revision 52
# baseline (speedup 1.0000x reference)
"""EqPBCNN (perturbation-based nonlinearity compensation NN) Trainium2 Bass kernel.

Data-parallel over 8 NeuronCores: batch 65536 -> 8192 per core.

Math (per sample):
  G_(a,b) = sum_q x[a,q] * conj(x[b,q])      (pol-independent; pairs (a,b)=(n+L, m+n+L))
  h1[p,o] = sum_m x[m,p] * R[p,o,m],  R = sum_n W1'[p,o,(m,n)] * G
  h2 = CLrelu(h1) @ W2^T; E = CLrelu(h2) @ W3^T
  out = x[center,p] + E * 10^(task0/10)/2

v3 design (engine-balanced rework of v2, ~132us -> ~115-120us):
  - conjugate-canonical pairs (148 of 175) folded into R weights (as v2);
    host pre-gathers pair stacks SAr/SAi/SBr/SBi [296, B/8] bf16 + SAD2/XP,
    stored chunk-major in DRAM ([nchunk*rows, 1024]).
  - P = 10^(task0/10) folded into XP on host (lrelu is positively
    homogeneous, so scaling the third factor x_m scales E linearly) -- the
    per-sample E*P multiply disappears.
  - DVE per chunk (1024 cols): 9 product mults + 1 merge add + 2 subtracts
    + 4 U mults, all bf16 SBUF (2x_1P mode, ~600ns each). Keeping ACT and
    GPSIMD quiet matters: concurrent ACT/GPSIMD SBUF traffic inflates every
    DVE op by ~20% (shared-port contention), so GPSIMD only does the OUT
    SWDGE and ACT only the R copies + lrelus.
  - PE per half (512 cols): 11 R matmuls (Pre0, P1k1, P2k1 w/ shared wg1,
    diag, Gim k0/k1 for re+im); per chunk: 8 fin (full [0:40] PSUM writes
    via zero-padded FINW blocks) + 1 w2 + 1 w3 (tail paired across the two
    halves via block-diagonal W2/W3).
  - 3-deep software pipeline: products(c+1) | R-matmuls(c) | U(c-2),
    fin(c-2), tail(c-3); loads 2 blocks ahead.
  - OUT layout [8, B/8/2]: rows 0:4 = half A (re0,re1,im0,im1), 4:8 half B;
    prefilled with center taps, E accumulated via SWDGE.
"""
import numpy as np
import ml_dtypes

BF16 = ml_dtypes.bfloat16

# ---------------- problem constants (hardcoded; must match reference) -------
BATCH = 65536
MT, LH = 41, 20          # filter taps, half window
NM = 2                   # modes / polarizations
H1, H2 = 2, 10
SLOPE = 0.01
NCORES = 8
BCORE = BATCH // NCORES  # 8192
NS = 1024                # columns per compute chunk
NCHUNK = BCORE // NS     # 8
LS = 1024                # columns per DMA load block (1 chunk)
HS = NS // 2             # 512-col half for PSUM/matmul stages

# ---------------- triplet / canonical-pair tables ---------------------------
_idx = [(m, n) for m in range(-LH, LH + 1) for n in range(-LH, LH + 1)
        if abs(m * n) <= LH and abs(m + n) <= LH and n >= m]
H = len(_idx)            # 175
SYM = np.where(np.array([m for m, n in _idx]) != np.array([n for m, n in _idx]),
               2.0, 1.0).astype(np.float32)
M_VALS = sorted(set(m for m, n in _idx))     # 25 distinct m values
NMV = len(M_VALS)
M_POS = {m: i for i, m in enumerate(M_VALS)}
NO = H1 * NMV * NM       # 100 rows of R/T space: (o, mi, p)

# canonical pairs: key (a,b) a<=b; triplet h -> (pair index, Gim sign)
_ckeys = {}
_tripmap = []
for (m, n) in _idx:
    a, b = n + LH, m + n + LH
    key, s = ((a, b), 1.0) if a <= b else ((b, a), -1.0)
    _ckeys[key] = None
    _tripmap.append((key, s))
POFF = sorted([k for k in _ckeys if k[0] < k[1]], key=lambda k: (k[1] - k[0], k[0]))
PDIAG = sorted([k for k in _ckeys if k[0] == k[1]])
NOFF, NDIAG = len(POFF), len(PDIAG)          # 127, 21
NPAIR = NOFF + NDIAG                          # 148
# stack rows: q0-off(127), q1-off(127), q0-diag(21), q1-diag(21)
NROWS_RE = 2 * NPAIR                          # 296 (P1/P2/Gre rows)
NROWS_IM = 2 * NOFF                           # 254 (P3/P4/Gim rows)
_pairpos = {}
for i, k in enumerate(POFF):
    _pairpos[k] = ('off', i)
for i, k in enumerate(PDIAG):
    _pairpos[k] = ('diag', i)


def _stack_row(kind, i, q):
    return q * NOFF + i if kind == 'off' else NROWS_IM + q * NDIAG + i


def _orow(o, mi, p):
    return (o * NMV + mi) * NM + p


def _hrow(p, o, comp):
    return (p * H1 + o) * 2 + comp


def _h2row(p, q, comp):
    return (p * H2 + q) * 2 + comp


# split boundaries of the 296-row stacks
KSP_RE = [(0, 128), (128, 128), (256, NROWS_RE - 256)]   # 128,128,40
KSP_IM = [(0, 128), (128, NROWS_IM - 128)]               # 128,126


def build_static():
    """Gather row tables (host side) + final/contraction constants."""
    # source rows within xq82 [2*41, BCORE] (rows = q*41 + tap)
    a_src = np.zeros(NROWS_RE, np.int64)
    b_src = np.zeros(NROWS_RE, np.int64)
    for key in POFF + PDIAG:
        kind, i = _pairpos[key]
        a, b = key
        for q in range(NM):
            r = _stack_row(kind, i, q)
            a_src[r] = q * MT + a
            b_src[r] = q * MT + b
    # xrep rows (o, mi, p) -> tap m+L, pol p
    xp_src = np.zeros(NO, np.int64)
    for o in range(H1):
        for mi, mv in enumerate(M_VALS):
            for p in range(NM):
                xp_src[_orow(o, mi, p)] = p * MT + (mv + LH)
    # final contraction: per u-group j (U1 +re, U2 -re, U3 +im, U4 +im), a
    # [100, 80] lhsT pair: cols 0:40 = half-A block (h1 rows 0:8, rest 0),
    # cols 40:80 = half-B block (h1 rows 32:40, rest 0). Zero columns make
    # every fin matmul write the full [0:40] PSUM range (no garbage rows).
    fin1 = np.zeros((NO, 4, 8), np.float32)
    for o in range(H1):
        for mi in range(NMV):
            for p in range(NM):
                r = _orow(o, mi, p)
                fin1[r, 0, _hrow(p, o, 0)] = 1.0
                fin1[r, 1, _hrow(p, o, 0)] = -1.0
                fin1[r, 2, _hrow(p, o, 1)] = 1.0
                fin1[r, 3, _hrow(p, o, 1)] = 1.0
    FINW = np.zeros((NO, 4, 2, 40), np.float32)
    FINW[:, :, 0, 0:8] = fin1
    FINW[:, :, 1, 32:40] = fin1
    FINW = FINW.reshape(NO, 320)
    d_src = np.concatenate([a_src[256:NROWS_RE], MT * NM + a_src[256:NROWS_RE]])
    return {"a_src": a_src, "b_src": b_src, "xp_src": xp_src, "d_src": d_src,
            "FINW": FINW}


def fold_weights(W1r, W1i, W2r, W2i, W3r, W3i):
    """Fold W1 (with SYM, pol-sum dup, conj-pair signs) into R-matmul lhsT."""
    Wr = W1r * SYM[None, None, :]   # [p, o, h]
    Wi = W1i * SYM[None, None, :]
    # WG [296, 200]: cols 0:100 -> Rre (+Wr), 100:200 -> Rim (+Wi); fed by Pre=P1+P2
    WG = np.zeros((NROWS_RE, 2 * NO), np.float32)
    # WI [254, 200]: Gim rows; cols 0:100 -> Rre (-s*Wi), 100:200 -> Rim (+s*Wr)
    WI = np.zeros((NROWS_IM, 2 * NO), np.float32)
    for h, (mn, (key, s)) in enumerate(zip(_idx, _tripmap)):
        m, n = mn
        kind, i = _pairpos[key]
        mi = M_POS[m]
        for p in range(NM):
            for o in range(H1):
                c = _orow(o, mi, p)
                for q in range(NM):
                    r = _stack_row(kind, i, q)
                    WG[r, c] += Wr[p, o, h]
                    WG[r, NO + c] += Wi[p, o, h]
                    if kind == 'off':
                        WI[r, c] += -s * Wi[p, o, h]
                        WI[r, NO + c] += s * Wr[p, o, h]
    WGP = np.zeros((3, 128, 2 * NO), np.float32)
    for k, (r0, rk) in enumerate(KSP_RE):
        WGP[k, :rk, :] = WG[r0:r0 + rk, :]
    WIP = np.zeros((2, 128, 2 * NO), np.float32)
    for k, (r0, rk) in enumerate(KSP_IM):
        WIP[k, :rk, :] = WI[r0:r0 + rk, :]
    WD = np.concatenate([WG[256:NROWS_RE, :], WG[256:NROWS_RE, :]], axis=0)
    # W2 lhsT [8, 40] on h1 rows (p,o,comp)
    W2L = np.zeros((8, 2 * H2 * NM), np.float32)
    for p in range(NM):
        for q in range(H2):
            for o in range(H1):
                W2L[_hrow(p, o, 0), _h2row(p, q, 0)] += W2r[p, q, o]
                W2L[_hrow(p, o, 1), _h2row(p, q, 0)] -= W2i[p, q, o]
                W2L[_hrow(p, o, 0), _h2row(p, q, 1)] += W2i[p, q, o]
                W2L[_hrow(p, o, 1), _h2row(p, q, 1)] += W2r[p, q, o]
    # W3 lhsT [40, 4]: out rows [re_p0, re_p1, im_p0, im_p1]; 1/NM folded
    W3L = np.zeros((2 * H2 * NM, 4), np.float32)
    s3 = 1.0 / NM
    for p in range(NM):
        for q in range(H2):
            W3L[_h2row(p, q, 0), 0 + p] += W3r[p, 0, q] * s3
            W3L[_h2row(p, q, 1), 0 + p] -= W3i[p, 0, q] * s3
            W3L[_h2row(p, q, 0), 2 + p] += W3i[p, 0, q] * s3
            W3L[_h2row(p, q, 1), 2 + p] += W3r[p, 0, q] * s3
    # paired-tail block-diagonal versions: half A -> h1 rows 0:8, E rows 0:4;
    # half B -> h1 rows 32:40 (PSUM col-tile position constraint), E rows 4:8
    W2L2 = np.zeros((40, 80), np.float32)
    W2L2[0:8, 0:40] = W2L
    W2L2[32:40, 40:80] = W2L
    W3L2 = np.zeros((80, 8), np.float32)
    W3L2[0:40, 0:4] = W3L
    W3L2[40:80, 4:8] = W3L
    return {"WGP": WGP, "WIP": WIP, "WD": WD, "W2L2": W2L2, "W3L2": W3L2}


# ---------------------------------------------------------------------------
def build_nc(bcore=BCORE, lrelu_mode="act"):
    """Build the Bass program for one core processing `bcore` samples."""
    import concourse.bass as bass
    import concourse.bacc as bacc
    import concourse.mybir as mybir
    from concourse.tile import TileContext
    import bass_rust

    nchunk = bcore // NS
    assert nchunk * NS == bcore and nchunk % 2 == 0
    f32 = mybir.dt.float32
    bf16 = mybir.dt.bfloat16
    AF = bass_rust.ActivationFunctionType
    OP = mybir.AluOpType

    nc = bacc.Bacc(None, target_bir_lowering=False, debug=False)
    # stack tensors are chunk-major: [nchunk * rows, NS] so each chunk's
    # block is one contiguous DRAM region (64KB DMA descriptors, not 2KB)
    saR = nc.declare_dram_parameter("SAr", [nchunk * NROWS_RE, NS], bf16, isOutput=False)
    saI = nc.declare_dram_parameter("SAi", [nchunk * NROWS_RE, NS], bf16, isOutput=False)
    sbR = nc.declare_dram_parameter("SBr", [nchunk * NROWS_RE, NS], bf16, isOutput=False)
    sbI = nc.declare_dram_parameter("SBi", [nchunk * NROWS_RE, NS], bf16, isOutput=False)
    sadD = nc.declare_dram_parameter("SAD2", [nchunk * 80, NS], bf16, isOutput=False)
    xpR = nc.declare_dram_parameter("XPr", [nchunk * NO, NS], bf16, isOutput=False)
    xpI = nc.declare_dram_parameter("XPi", [nchunk * NO, NS], bf16, isOutput=False)
    ctrD = nc.declare_dram_parameter("CTR2", [8, bcore // 2], f32, isOutput=False)
    wgD = nc.declare_dram_parameter("WGP", [3, 128, 2 * NO], bf16, isOutput=False)
    wiD = nc.declare_dram_parameter("WIP", [2, 128, 2 * NO], bf16, isOutput=False)
    wdD = nc.declare_dram_parameter("WD", [80, 2 * NO], bf16, isOutput=False)
    finD = nc.declare_dram_parameter("FINW", [NO, 320], bf16, isOutput=False)
    w2D = nc.declare_dram_parameter("W2L2", [40, 80], bf16, isOutput=False)
    w3D = nc.declare_dram_parameter("W3L2", [80, 8], bf16, isOutput=False)
    outD = nc.declare_dram_parameter("OUT2", [8, bcore // 2], f32, isOutput=True)

    with TileContext(nc) as tc:
        with (
            tc.tile_pool(name="consts", bufs=1) as cp,
            tc.tile_pool(name="mega", bufs=3) as mp,
            tc.tile_pool(name="xpp", bufs=5) as xpp,
            tc.tile_pool(name="prod", bufs=2) as up,
            tc.tile_pool(name="rcopy", bufs=3) as rp,
            tc.tile_pool(name="tt", bufs=2) as tp,
            tc.tile_pool(name="psumr", bufs=4, space="PSUM") as ppr,
            tc.tile_pool(name="psum1", bufs=2, space="PSUM") as pp1,
            tc.tile_pool(name="psum2", bufs=1, space="PSUM") as pp2,
            tc.tile_pool(name="psume", bufs=1, space="PSUM") as ppe,
        ):
            def const_tile(src_ap, name):
                # consts go through the gpsimd SWDGE queue: its completion
                # semaphore is separate from the HWDGE load queues, so the
                # first R matmul doesn't wait behind block loads
                tr = cp.tile(list(src_ap.shape), bf16, name=name)
                nc.gpsimd.dma_start(out=tr[:], in_=src_ap)
                return tr

            wg_sb, wi_sb = [None] * 3, [None] * 2
            fw = {}

            def emit_consts():
                for k in range(3):
                    wg_sb[k] = const_tile(wgD[k], f"wg{k}")
                for k in range(2):
                    wi_sb[k] = const_tile(wiD[k], f"wi{k}")
                fw["wd"] = const_tile(wdD[:], "wd")
                fw["fin"] = const_tile(finD[:], "fin")
                fw["w2"] = const_tile(w2D[:], "w2")
                fw["w3"] = const_tile(w3D[:], "w3")

            stage = {}           # c -> product tiles etc for chunk c
            lstage = {}          # lb -> loaded stack tiles (2048-col)
            rstage = {}          # c -> (rre_s, rim_s) [100, 1024] bf16
            ustage = {}          # c -> u tiles
            tstage = {}          # c -> tail intermediates

            def lrelu(dst, src_ap):
                if lrelu_mode == "act":
                    nc.scalar.activation(dst, src_ap, AF.Lrelu, alpha=SLOPE)
                else:
                    nc.vector.tensor_scalar_mul(dst, src_ap, SLOPE)
                    nc.vector.tensor_tensor(dst, dst, src_ap, op=OP.max)

            def load_block(lb):
                sa_r, sa_i, sb_r, sb_i = [], [], [], []
                for k, (r0, rk) in enumerate(KSP_RE[:2]):
                    for nm_, src_, lst, eng in (
                        (f"sar{k}", saR, sa_r, nc.sync), (f"sai{k}", saI, sa_i, nc.sync),
                        (f"sbr{k}", sbR, sb_r, nc.scalar), (f"sbi{k}", sbI, sb_i, nc.scalar)):
                        t = mp.tile([rk, LS], bf16, tag=nm_)
                        b0 = lb * NROWS_RE + r0
                        eng.dma_start(out=t[:], in_=src_[b0:b0 + rk, :])
                        lst.append(t)
                sad = mp.tile([80, LS], bf16, tag="sad")
                nc.sync.dma_start(out=sad[:], in_=sadD[lb * 80:(lb + 1) * 80, :])
                xp_r = xpp.tile([NO, LS], bf16, tag="xpr")
                xp_i = xpp.tile([NO, LS], bf16, tag="xpi")
                nc.sync.dma_start(out=xp_r[:], in_=xpR[lb * NO:(lb + 1) * NO, :])
                nc.scalar.dma_start(out=xp_i[:], in_=xpI[lb * NO:(lb + 1) * NO, :])
                lstage[lb] = (sa_r, sa_i, sb_r, sb_i, sad, xp_r, xp_i)

            def stage_a(c):
                # products for chunk c from load block c
                sa_r, sa_i, sb_r, sb_i, sad, xp_r, xp_i = lstage[c]
                qs = slice(0, NS)
                pre, sub = [], []
                # P1, P2 products; k=0 pair merged on DVE, k=1 pair left
                # separate (PE accumulates both with the same wg1 weights)
                for k in range(2):
                    t1 = up.tile([128, NS], bf16, tag=f"p1_{k}")
                    t2 = up.tile([128, NS], bf16, tag=f"p2_{k}")
                    nc.vector.tensor_tensor(t1[:], sa_r[k][:, qs], sb_r[k][:, qs], op=OP.mult)
                    nc.vector.tensor_tensor(t2[:], sa_i[k][:, qs], sb_i[k][:, qs], op=OP.mult)
                    if k == 0:
                        nc.vector.tensor_tensor(t1[:], t1[:], t2[:], op=OP.add)
                        pre.append(t1)
                    else:
                        pre.append(t1)
                        pre.append(t2)
                pd = up.tile([80, NS], bf16, tag="pd")
                nc.scalar.square(pd[:], sad[:, qs])
                # P3 - P4 for k=0,1 (Gim feed)
                for k, (r0, rk) in enumerate(KSP_IM):
                    t3 = up.tile([rk, NS], bf16, tag=f"p3_{k}")
                    t4 = up.tile([rk, NS], bf16, tag=f"p4_{k}")
                    nc.vector.tensor_tensor(t3[:], sa_i[k][:rk, qs], sb_r[k][:rk, qs], op=OP.mult)
                    nc.vector.tensor_tensor(t4[:], sa_r[k][:rk, qs], sb_i[k][:rk, qs], op=OP.mult)
                    nc.vector.tensor_tensor(t3[:], t3[:], t4[:], op=OP.subtract)
                    sub.append(t3)
                stage[c] = (pre, pd, sub, xp_r, xp_i)

            def stage_b(c, h):
                # R matmuls for one 512-col half: 5 feeds x {rre, rim}
                pre, pd, sub, xp_r, xp_i = stage[c]
                hs = slice(h * HS, (h + 1) * HS)
                p_rre = ppr.tile([128, HS], f32, tag="pr")
                p_rim = ppr.tile([128, HS], f32, tag="pr")
                for j, (wk, t) in enumerate(((0, pre[0]), (1, pre[1]), (1, pre[2]))):
                    wg = wg_sb[wk]
                    st = (j == 0)
                    nc.tensor.matmul(p_rre[:NO], wg[:128, 0:NO], t[:, hs], start=st, stop=False)
                    nc.tensor.matmul(p_rim[:NO], wg[:128, NO:2 * NO], t[:, hs], start=st, stop=False)
                wd = fw["wd"]
                nc.tensor.matmul(p_rre[:NO], wd[:, 0:NO], pd[:, hs], start=False, stop=False)
                nc.tensor.matmul(p_rim[:NO], wd[:, NO:2 * NO], pd[:, hs], start=False, stop=False)
                for k, (r0, rk) in enumerate(KSP_IM):
                    wi = wi_sb[k]
                    sp = (k == 1)
                    nc.tensor.matmul(p_rre[:NO], wi[:rk, 0:NO], sub[k][:, hs], start=False, stop=sp)
                    nc.tensor.matmul(p_rim[:NO], wi[:rk, NO:2 * NO], sub[k][:, hs], start=False, stop=sp)
                rstage[(c, h)] = (p_rre, p_rim)

            def copy_r(c, h):
                # PSUM -> SBUF bf16 into the full-chunk R tiles
                if h == 0:
                    rre = rp.tile([NO, NS], bf16, tag="rres")
                    rim = rp.tile([NO, NS], bf16, tag="rims")
                    rstage[c] = (rre, rim)
                rre, rim = rstage[c]
                p_rre, p_rim = rstage.pop((c, h))
                hs = slice(h * HS, (h + 1) * HS)
                nc.scalar.copy(rre[:, hs], p_rre[:NO])
                nc.scalar.copy(rim[:, hs], p_rim[:NO])

            def stage_u(c):
                # T products U = XP * R on the full 1024-col chunk (DVE)
                pre, pd, sub, xp_r, xp_i = stage.pop(c)
                rre, rim = rstage.pop(c)
                qs = slice(0, NS)
                lstage.pop(c, None)
                u1 = tp.tile([NO, NS], bf16, tag="u1")
                u2 = tp.tile([NO, NS], bf16, tag="u2")
                u3 = tp.tile([NO, NS], bf16, tag="u3")
                u4 = tp.tile([NO, NS], bf16, tag="u4")
                nc.vector.tensor_tensor(u1[:], xp_r[:, qs], rre[:], op=OP.mult)
                nc.vector.tensor_tensor(u2[:], xp_i[:, qs], rim[:], op=OP.mult)
                nc.vector.tensor_tensor(u3[:], xp_r[:, qs], rim[:], op=OP.mult)
                nc.vector.tensor_tensor(u4[:], xp_i[:, qs], rre[:], op=OP.mult)
                ustage[c] = (u1, u2, u3, u4)

            def stage_fin(c):
                # fin contraction for both halves into one full [40, HS] PSUM
                # tile: half A lands in rows 0:8, half B in rows 32:40, zeros
                # elsewhere (the zero weight columns write them explicitly)
                us = ustage.pop(c)
                fin_sb = fw["fin"]
                p_h1 = pp1.tile([40, HS], f32, tag="ph1")
                for h in (0, 1):
                    hs = slice(h * HS, (h + 1) * HS)
                    for j in range(4):
                        c0 = j * 80 + h * 40
                        nc.tensor.matmul(p_h1[0:40], fin_sb[:, c0:c0 + 40], us[j][:, hs],
                                         start=(h == 0 and j == 0), stop=(h == 1 and j == 3))
                h1s = tp.tile([40, HS], bf16, tag="h1s")
                lrelu(h1s[:], p_h1[0:40])
                tstage[c] = h1s

            def stage_tail(c):
                # paired MLP tail: w2 -> lrelu -> w3 -> E copy -> OUT accum
                h1s = tstage.pop(c)
                p_h2 = pp2.tile([80, HS], f32, tag="ph2")
                nc.tensor.matmul(p_h2[:80], fw["w2"][:], h1s[:], start=True, stop=True)
                h2s = tp.tile([80, HS], bf16, tag="h2s")
                lrelu(h2s[:], p_h2[:80])
                p_e = ppe.tile([8, HS], f32, tag="pe")
                nc.tensor.matmul(p_e[:8], fw["w3"][:], h2s[:], start=True, stop=True)
                eab = tp.tile([8, HS], f32, tag="eab")
                nc.scalar.copy(eab[:], p_e[:8])
                cs = slice(c * HS, (c + 1) * HS)
                nc.gpsimd.dma_start(out=outD[:, cs], in_=eab[:], accum_op=OP.add)

            # ---------------- schedule ----------------
            # consts first (gpsimd SWDGE, absorbs the one-time Q7 IRAM
            # load); block 0 leads the HWDGE queues so the first products
            # start as early as possible
            emit_consts()
            load_block(0)
            stage_a(0)
            # pre-fill OUT with the center taps; E accumulates onto it
            nc.gpsimd.dma_start(out=outD[:, :], in_=ctrD[:, :])
            load_block(1)
            for c in range(nchunk):
                if c >= 2:
                    stage_u(c - 2)
                if c + 2 < nchunk:
                    load_block(c + 2)
                # R matmuls for chunk c BEFORE emitting products(c+1): keeps
                # PE's semaphore waits scoped to already-finished DVE work
                stage_b(c, 0)
                copy_r(c, 0)
                stage_b(c, 1)
                copy_r(c, 1)
                if c + 1 < nchunk:
                    stage_a(c + 1)
                if c >= 2:
                    stage_fin(c - 2)
                if c >= 3:
                    stage_tail(c - 3)
            stage_u(nchunk - 2)
            stage_fin(nchunk - 2)
            stage_tail(nchunk - 3)
            stage_u(nchunk - 1)
            stage_fin(nchunk - 1)
            stage_tail(nchunk - 2)
            stage_tail(nchunk - 1)
    nc.compile()
    return nc


# ---------------------------------------------------------------------------
def _prep_core_inputs(inputs, static, folded):
    """Host-side gather + shard. Returns list of per-core in_maps."""
    xr = np.asarray(inputs["x_real"])     # [B, 41, 2]
    xi = np.asarray(inputs["x_imag"])
    t0 = np.ascontiguousarray(np.asarray(inputs["task_info"])[:, 0])
    # xq82 rows = q*41 + tap
    xrq = np.ascontiguousarray(xr.transpose(2, 1, 0).reshape(2 * MT, BATCH))
    xiq = np.ascontiguousarray(xi.transpose(2, 1, 0).reshape(2 * MT, BATCH))
    a_src, b_src, xp_src = static["a_src"], static["b_src"], static["xp_src"]
    xri = np.concatenate([xrq, xiq], axis=0)
    SAD2 = xri[static["d_src"]].astype(BF16)
    SAr = xrq[a_src].astype(BF16)
    SAi = xiq[a_src].astype(BF16)
    SBr = xrq[b_src].astype(BF16)
    SBi = xiq[b_src].astype(BF16)
    # P = 10^(t0/10) folded into the third-factor replicas (lrelu is
    # positively homogeneous; 1/NM stays folded in W3L)
    pex = (10.0 ** (t0[None, :] / 10.0)).astype(np.float32)
    XPr = (xrq[xp_src] * pex).astype(BF16)
    XPi = (xiq[xp_src] * pex).astype(BF16)
    # CTR2 [8, BATCH/2]: per chunk c, cols [c*HS,(c+1)*HS): rows 0:4 = half A
    # (samples c*NS..c*NS+HS), rows 4:8 = half B (c*NS+HS..(c+1)*NS)
    ctr4 = np.stack([xrq[LH], xrq[MT + LH], xiq[LH], xiq[MT + LH]], axis=0)
    ctr2 = ctr4.reshape(4, BATCH // NS, 2, HS).transpose(2, 0, 1, 3).reshape(8, BATCH // 2)
    shared = {"WGP": folded["WGP"].astype(BF16), "WIP": folded["WIP"].astype(BF16),
              "WD": folded["WD"].astype(BF16), "FINW": static["FINW"].astype(BF16),
              "W2L2": folded["W2L2"].astype(BF16), "W3L2": folded["W3L2"].astype(BF16)}
    def chunk_major(a, s):
        # [rows, BCORE] core slice -> [nchunk*rows, NS] chunk-major
        rows = a.shape[0]
        return np.ascontiguousarray(
            a[:, s].reshape(rows, NCHUNK, NS).transpose(1, 0, 2).reshape(NCHUNK * rows, NS))

    in_maps = []
    hc = BCORE // 2
    for c in range(NCORES):
        s = slice(c * BCORE, (c + 1) * BCORE)
        s2 = slice(c * hc, (c + 1) * hc)
        m = dict(shared)
        m["SAr"] = chunk_major(SAr, s)
        m["SAi"] = chunk_major(SAi, s)
        m["SBr"] = chunk_major(SBr, s)
        m["SBi"] = chunk_major(SBi, s)
        m["SAD2"] = chunk_major(SAD2, s)
        m["XPr"] = chunk_major(XPr, s)
        m["XPi"] = chunk_major(XPi, s)
        m["CTR2"] = np.ascontiguousarray(ctr2[:, s2])
        in_maps.append(m)
    return in_maps


def unshuffle_out2(o8, bcore=BCORE):
    """OUT2 [8, bcore/2] -> [bcore, 2, 2] (sample, pol, re/im)."""
    nch = bcore // NS
    o = o8.reshape(2, 4, nch, HS)          # (half, comp, chunk, col)
    out = np.empty((bcore, NM, 2), np.float32)
    flat = o.transpose(2, 0, 3, 1).reshape(bcore, 4)   # (chunk, half, col, comp)
    out[:, 0, 0] = flat[:, 0]
    out[:, 1, 0] = flat[:, 1]
    out[:, 0, 1] = flat[:, 2]
    out[:, 1, 1] = flat[:, 3]
    return out


_CACHE = {}


def kernel(**inputs):
    from concourse.bass_utils import run_bass_kernel_spmd

    static = build_static()
    folded = fold_weights(
        np.asarray(inputs["W1_real"]), np.asarray(inputs["W1_imag"]),
        np.asarray(inputs["W2_real"]), np.asarray(inputs["W2_imag"]),
        np.asarray(inputs["W3_real"]), np.asarray(inputs["W3_imag"]),
    )
    if "nc" not in _CACHE:
        _CACHE["nc"] = build_nc()
    nc = _CACHE["nc"]
    in_maps = _prep_core_inputs(inputs, static, folded)
    res = run_bass_kernel_spmd(nc, in_maps, list(range(NCORES)))
    out = np.empty((BATCH, NM, 2), np.float32)
    for c in range(NCORES):
        o8 = res.results[c]["OUT2"]
        s = slice(c * BCORE, (c + 1) * BCORE)
        out[s] = unshuffle_out2(o8)
    return out


# revision 53
# speedup vs baseline: 1.0163x; 1.0163x over previous
"""EqPBCNN (perturbation-based nonlinearity compensation NN) Trainium2 Bass kernel.

Data-parallel over 8 NeuronCores: batch 65536 -> 8192 per core.

Math (per sample):
  G_(a,b) = sum_q x[a,q] * conj(x[b,q])      (pol-independent; pairs (a,b)=(n+L, m+n+L))
  h1[p,o] = sum_m x[m,p] * R[p,o,m],  R = sum_n W1'[p,o,(m,n)] * G
  h2 = CLrelu(h1) @ W2^T; E = CLrelu(h2) @ W3^T
  out = x[center,p] + E * 10^(task0/10)/2

v3 design (engine-balanced rework of v2, ~132us -> ~115-120us):
  - conjugate-canonical pairs (148 of 175) folded into R weights (as v2);
    host pre-gathers pair stacks SAr/SAi/SBr/SBi [296, B/8] bf16 + SAD2/XP,
    stored chunk-major in DRAM ([nchunk*rows, 1024]).
  - P = 10^(task0/10) folded into XP on host (lrelu is positively
    homogeneous, so scaling the third factor x_m scales E linearly) -- the
    per-sample E*P multiply disappears.
  - DVE per chunk (1024 cols): 9 product mults + 1 merge add + 2 subtracts
    + 4 U mults, all bf16 SBUF (2x_1P mode, ~600ns each). Keeping ACT and
    GPSIMD quiet matters: concurrent ACT/GPSIMD SBUF traffic inflates every
    DVE op by ~20% (shared-port contention), so GPSIMD only does the OUT
    SWDGE and ACT only the R copies + lrelus.
  - PE per half (512 cols): 11 R matmuls (Pre0, P1k1, P2k1 w/ shared wg1,
    diag, Gim k0/k1 for re+im); per chunk: 8 fin (full [0:40] PSUM writes
    via zero-padded FINW blocks) + 1 w2 + 1 w3 (tail paired across the two
    halves via block-diagonal W2/W3).
  - 3-deep software pipeline: products(c+1) | R-matmuls(c) | U(c-2),
    fin(c-2), tail(c-3); loads 2 blocks ahead.
  - OUT layout [8, B/8/2]: rows 0:4 = half A (re0,re1,im0,im1), 4:8 half B;
    prefilled with center taps, E accumulated via SWDGE.
"""
import numpy as np
import ml_dtypes

BF16 = ml_dtypes.bfloat16

# ---------------- problem constants (hardcoded; must match reference) -------
BATCH = 65536
MT, LH = 41, 20          # filter taps, half window
NM = 2                   # modes / polarizations
H1, H2 = 2, 10
SLOPE = 0.01
NCORES = 8
BCORE = BATCH // NCORES  # 8192
NS = 1024                # columns per compute chunk
NCHUNK = BCORE // NS     # 8
LS = 1024                # columns per DMA load block (1 chunk)
HS = NS // 2             # 512-col half for PSUM/matmul stages

# ---------------- triplet / canonical-pair tables ---------------------------
_idx = [(m, n) for m in range(-LH, LH + 1) for n in range(-LH, LH + 1)
        if abs(m * n) <= LH and abs(m + n) <= LH and n >= m]
H = len(_idx)            # 175
SYM = np.where(np.array([m for m, n in _idx]) != np.array([n for m, n in _idx]),
               2.0, 1.0).astype(np.float32)
M_VALS = sorted(set(m for m, n in _idx))     # 25 distinct m values
NMV = len(M_VALS)
M_POS = {m: i for i, m in enumerate(M_VALS)}
NO = H1 * NMV * NM       # 100 rows of R/T space: (o, mi, p)

# canonical pairs: key (a,b) a<=b; triplet h -> (pair index, Gim sign)
_ckeys = {}
_tripmap = []
for (m, n) in _idx:
    a, b = n + LH, m + n + LH
    key, s = ((a, b), 1.0) if a <= b else ((b, a), -1.0)
    _ckeys[key] = None
    _tripmap.append((key, s))
POFF = sorted([k for k in _ckeys if k[0] < k[1]], key=lambda k: (k[1] - k[0], k[0]))
PDIAG = sorted([k for k in _ckeys if k[0] == k[1]])
NOFF, NDIAG = len(POFF), len(PDIAG)          # 127, 21
NPAIR = NOFF + NDIAG                          # 148
# stack rows: q0-off(127), q1-off(127), q0-diag(21), q1-diag(21)
NROWS_RE = 2 * NPAIR                          # 296 (P1/P2/Gre rows)
NROWS_IM = 2 * NOFF                           # 254 (P3/P4/Gim rows)
_pairpos = {}
for i, k in enumerate(POFF):
    _pairpos[k] = ('off', i)
for i, k in enumerate(PDIAG):
    _pairpos[k] = ('diag', i)


def _stack_row(kind, i, q):
    return q * NOFF + i if kind == 'off' else NROWS_IM + q * NDIAG + i


def _orow(o, mi, p):
    return (o * NMV + mi) * NM + p


def _hrow(p, o, comp):
    return (p * H1 + o) * 2 + comp


def _h2row(p, q, comp):
    return (p * H2 + q) * 2 + comp


# split boundaries of the 296-row stacks
KSP_RE = [(0, 128), (128, 128), (256, NROWS_RE - 256)]   # 128,128,40
KSP_IM = [(0, 128), (128, NROWS_IM - 128)]               # 128,126


def build_static():
    """Gather row tables (host side) + final/contraction constants."""
    # source rows within xq82 [2*41, BCORE] (rows = q*41 + tap)
    a_src = np.zeros(NROWS_RE, np.int64)
    b_src = np.zeros(NROWS_RE, np.int64)
    for key in POFF + PDIAG:
        kind, i = _pairpos[key]
        a, b = key
        for q in range(NM):
            r = _stack_row(kind, i, q)
            a_src[r] = q * MT + a
            b_src[r] = q * MT + b
    # xrep rows (o, mi, p) -> tap m+L, pol p
    xp_src = np.zeros(NO, np.int64)
    for o in range(H1):
        for mi, mv in enumerate(M_VALS):
            for p in range(NM):
                xp_src[_orow(o, mi, p)] = p * MT + (mv + LH)
    # final contraction: per u-group j (U1 +re, U2 -re, U3 +im, U4 +im), a
    # [100, 80] lhsT pair: cols 0:40 = half-A block (h1 rows 0:8, rest 0),
    # cols 40:80 = half-B block (h1 rows 32:40, rest 0). Zero columns make
    # every fin matmul write the full [0:40] PSUM range (no garbage rows).
    fin1 = np.zeros((NO, 4, 8), np.float32)
    for o in range(H1):
        for mi in range(NMV):
            for p in range(NM):
                r = _orow(o, mi, p)
                fin1[r, 0, _hrow(p, o, 0)] = 1.0
                fin1[r, 1, _hrow(p, o, 0)] = -1.0
                fin1[r, 2, _hrow(p, o, 1)] = 1.0
                fin1[r, 3, _hrow(p, o, 1)] = 1.0
    FINW = np.zeros((NO, 4, 2, 40), np.float32)
    FINW[:, :, 0, 0:8] = fin1
    FINW[:, :, 1, 32:40] = fin1
    FINW = FINW.reshape(NO, 320)
    d_src = np.concatenate([a_src[256:NROWS_RE], MT * NM + a_src[256:NROWS_RE]])
    return {"a_src": a_src, "b_src": b_src, "xp_src": xp_src, "d_src": d_src,
            "FINW": FINW}


def fold_weights(W1r, W1i, W2r, W2i, W3r, W3i):
    """Fold W1 (with SYM, pol-sum dup, conj-pair signs) into R-matmul lhsT."""
    Wr = W1r * SYM[None, None, :]   # [p, o, h]
    Wi = W1i * SYM[None, None, :]
    # WG [296, 200]: cols 0:100 -> Rre (+Wr), 100:200 -> Rim (+Wi); fed by Pre=P1+P2
    WG = np.zeros((NROWS_RE, 2 * NO), np.float32)
    # WI [254, 200]: Gim rows; cols 0:100 -> Rre (-s*Wi), 100:200 -> Rim (+s*Wr)
    WI = np.zeros((NROWS_IM, 2 * NO), np.float32)
    for h, (mn, (key, s)) in enumerate(zip(_idx, _tripmap)):
        m, n = mn
        kind, i = _pairpos[key]
        mi = M_POS[m]
        for p in range(NM):
            for o in range(H1):
                c = _orow(o, mi, p)
                for q in range(NM):
                    r = _stack_row(kind, i, q)
                    WG[r, c] += Wr[p, o, h]
                    WG[r, NO + c] += Wi[p, o, h]
                    if kind == 'off':
                        WI[r, c] += -s * Wi[p, o, h]
                        WI[r, NO + c] += s * Wr[p, o, h]
    WGP = np.zeros((3, 128, 2 * NO), np.float32)
    for k, (r0, rk) in enumerate(KSP_RE):
        WGP[k, :rk, :] = WG[r0:r0 + rk, :]
    WIP = np.zeros((2, 128, 2 * NO), np.float32)
    for k, (r0, rk) in enumerate(KSP_IM):
        WIP[k, :rk, :] = WI[r0:r0 + rk, :]
    WD = np.concatenate([WG[256:NROWS_RE, :], WG[256:NROWS_RE, :]], axis=0)
    # W2 lhsT [8, 40] on h1 rows (p,o,comp)
    W2L = np.zeros((8, 2 * H2 * NM), np.float32)
    for p in range(NM):
        for q in range(H2):
            for o in range(H1):
                W2L[_hrow(p, o, 0), _h2row(p, q, 0)] += W2r[p, q, o]
                W2L[_hrow(p, o, 1), _h2row(p, q, 0)] -= W2i[p, q, o]
                W2L[_hrow(p, o, 0), _h2row(p, q, 1)] += W2i[p, q, o]
                W2L[_hrow(p, o, 1), _h2row(p, q, 1)] += W2r[p, q, o]
    # W3 lhsT [40, 4]: out rows [re_p0, re_p1, im_p0, im_p1]; 1/NM folded
    W3L = np.zeros((2 * H2 * NM, 4), np.float32)
    s3 = 1.0 / NM
    for p in range(NM):
        for q in range(H2):
            W3L[_h2row(p, q, 0), 0 + p] += W3r[p, 0, q] * s3
            W3L[_h2row(p, q, 1), 0 + p] -= W3i[p, 0, q] * s3
            W3L[_h2row(p, q, 0), 2 + p] += W3i[p, 0, q] * s3
            W3L[_h2row(p, q, 1), 2 + p] += W3r[p, 0, q] * s3
    # paired-tail block-diagonal versions: half A -> h1 rows 0:8, E rows 0:4;
    # half B -> h1 rows 32:40 (PSUM col-tile position constraint), E rows 4:8
    W2L2 = np.zeros((40, 80), np.float32)
    W2L2[0:8, 0:40] = W2L
    W2L2[32:40, 40:80] = W2L
    W3L2 = np.zeros((80, 8), np.float32)
    W3L2[0:40, 0:4] = W3L
    W3L2[40:80, 4:8] = W3L
    return {"WGP": WGP, "WIP": WIP, "WD": WD, "W2L2": W2L2, "W3L2": W3L2}


# ---------------------------------------------------------------------------
def build_nc(bcore=BCORE, lrelu_mode="act"):
    """Build the Bass program for one core processing `bcore` samples."""
    import concourse.bass as bass
    import concourse.bacc as bacc
    import concourse.mybir as mybir
    from concourse.tile import TileContext
    import bass_rust

    nchunk = bcore // NS
    assert nchunk * NS == bcore and nchunk % 2 == 0
    f32 = mybir.dt.float32
    bf16 = mybir.dt.bfloat16
    AF = bass_rust.ActivationFunctionType
    OP = mybir.AluOpType

    nc = bacc.Bacc(None, target_bir_lowering=False, debug=False)
    # stack tensors are chunk-major: [nchunk * rows, NS] so each chunk's
    # block is one contiguous DRAM region (64KB DMA descriptors, not 2KB)
    saR = nc.declare_dram_parameter("SAr", [nchunk * NROWS_RE, NS], bf16, isOutput=False)
    saI = nc.declare_dram_parameter("SAi", [nchunk * NROWS_RE, NS], bf16, isOutput=False)
    sbR = nc.declare_dram_parameter("SBr", [nchunk * NROWS_RE, NS], bf16, isOutput=False)
    sbI = nc.declare_dram_parameter("SBi", [nchunk * NROWS_RE, NS], bf16, isOutput=False)
    sadD = nc.declare_dram_parameter("SAD2", [nchunk * 80, NS], bf16, isOutput=False)
    xpR = nc.declare_dram_parameter("XPr", [nchunk * NO, NS], bf16, isOutput=False)
    xpI = nc.declare_dram_parameter("XPi", [nchunk * NO, NS], bf16, isOutput=False)
    ctrD = nc.declare_dram_parameter("CTR2", [8, bcore // 2], f32, isOutput=False)
    wgD = nc.declare_dram_parameter("WGP", [3, 128, 2 * NO], bf16, isOutput=False)
    wiD = nc.declare_dram_parameter("WIP", [2, 128, 2 * NO], bf16, isOutput=False)
    wdD = nc.declare_dram_parameter("WD", [80, 2 * NO], bf16, isOutput=False)
    finD = nc.declare_dram_parameter("FINW", [NO, 320], bf16, isOutput=False)
    w2D = nc.declare_dram_parameter("W2L2", [40, 80], bf16, isOutput=False)
    w3D = nc.declare_dram_parameter("W3L2", [80, 8], bf16, isOutput=False)
    outD = nc.declare_dram_parameter("OUT2", [8, bcore // 2], f32, isOutput=True)

    with TileContext(nc) as tc:
        with (
            tc.tile_pool(name="consts", bufs=1) as cp,
            tc.tile_pool(name="mega", bufs=3) as mp,
            tc.tile_pool(name="xpp", bufs=5) as xpp,
            tc.tile_pool(name="prod", bufs=2) as up,
            tc.tile_pool(name="rcopy", bufs=3) as rp,
            tc.tile_pool(name="tt", bufs=2) as tp,
            tc.tile_pool(name="psumr", bufs=4, space="PSUM") as ppr,
            tc.tile_pool(name="psum1", bufs=2, space="PSUM") as pp1,
            tc.tile_pool(name="psum2", bufs=1, space="PSUM") as pp2,
            tc.tile_pool(name="psume", bufs=1, space="PSUM") as ppe,
        ):
            def const_tile(src_ap, name):
                # consts go through the gpsimd SWDGE queue: its completion
                # semaphore is separate from the HWDGE load queues, so the
                # first R matmul doesn't wait behind block loads
                tr = cp.tile(list(src_ap.shape), bf16, name=name)
                nc.gpsimd.dma_start(out=tr[:], in_=src_ap)
                return tr

            wg_sb, wi_sb = [None] * 3, [None] * 2
            fw = {}

            def emit_consts():
                for k in range(3):
                    wg_sb[k] = const_tile(wgD[k], f"wg{k}")
                for k in range(2):
                    wi_sb[k] = const_tile(wiD[k], f"wi{k}")
                fw["wd"] = const_tile(wdD[:], "wd")
                fw["fin"] = const_tile(finD[:], "fin")
                fw["w2"] = const_tile(w2D[:], "w2")
                fw["w3"] = const_tile(w3D[:], "w3")

            stage = {}           # c -> product tiles etc for chunk c
            lstage = {}          # lb -> loaded stack tiles (2048-col)
            rstage = {}          # c -> (rre_s, rim_s) [100, 1024] bf16
            ustage = {}          # c -> u tiles
            tstage = {}          # c -> tail intermediates

            def lrelu(dst, src_ap):
                if lrelu_mode == "act":
                    nc.scalar.activation(dst, src_ap, AF.Lrelu, alpha=SLOPE)
                else:
                    nc.vector.tensor_scalar_mul(dst, src_ap, SLOPE)
                    nc.vector.tensor_tensor(dst, dst, src_ap, op=OP.max)

            def load_block(lb):
                sa_r, sa_i, sb_r, sb_i = [], [], [], []
                for k, (r0, rk) in enumerate(KSP_RE[:2]):
                    for nm_, src_, lst, eng in (
                        (f"sar{k}", saR, sa_r, nc.sync), (f"sai{k}", saI, sa_i, nc.sync),
                        (f"sbr{k}", sbR, sb_r, nc.scalar), (f"sbi{k}", sbI, sb_i, nc.scalar)):
                        t = mp.tile([rk, LS], bf16, tag=nm_)
                        b0 = lb * NROWS_RE + r0
                        eng.dma_start(out=t[:], in_=src_[b0:b0 + rk, :])
                        lst.append(t)
                sad = mp.tile([80, LS], bf16, tag="sad")
                nc.sync.dma_start(out=sad[:], in_=sadD[lb * 80:(lb + 1) * 80, :])
                xp_r = xpp.tile([NO, LS], bf16, tag="xpr")
                xp_i = xpp.tile([NO, LS], bf16, tag="xpi")
                nc.sync.dma_start(out=xp_r[:], in_=xpR[lb * NO:(lb + 1) * NO, :])
                nc.scalar.dma_start(out=xp_i[:], in_=xpI[lb * NO:(lb + 1) * NO, :])
                lstage[lb] = (sa_r, sa_i, sb_r, sb_i, sad, xp_r, xp_i)

            def stage_a(c):
                # products for chunk c from load block c
                sa_r, sa_i, sb_r, sb_i, sad, xp_r, xp_i = lstage[c]
                qs = slice(0, NS)
                pre, sub = [], []
                # P1, P2 products; k=0 pair merged on DVE, k=1 pair left
                # separate (PE accumulates both with the same wg1 weights)
                for k in range(2):
                    t1 = up.tile([128, NS], bf16, tag=f"p1_{k}")
                    t2 = up.tile([128, NS], bf16, tag=f"p2_{k}")
                    nc.vector.tensor_tensor(t1[:], sa_r[k][:, qs], sb_r[k][:, qs], op=OP.mult)
                    nc.vector.tensor_tensor(t2[:], sa_i[k][:, qs], sb_i[k][:, qs], op=OP.mult)
                    if k == 0:
                        nc.vector.tensor_tensor(t1[:], t1[:], t2[:], op=OP.add)
                        pre.append(t1)
                    else:
                        pre.append(t1)
                        pre.append(t2)
                pd = up.tile([80, NS], bf16, tag="pd")
                nc.vector.tensor_tensor(pd[:], sad[:, qs], sad[:, qs], op=OP.mult)
                # P3 - P4 for k=0,1 (Gim feed)
                for k, (r0, rk) in enumerate(KSP_IM):
                    t3 = up.tile([rk, NS], bf16, tag=f"p3_{k}")
                    t4 = up.tile([rk, NS], bf16, tag=f"p4_{k}")
                    nc.vector.tensor_tensor(t3[:], sa_i[k][:rk, qs], sb_r[k][:rk, qs], op=OP.mult)
                    nc.vector.tensor_tensor(t4[:], sa_r[k][:rk, qs], sb_i[k][:rk, qs], op=OP.mult)
                    nc.vector.tensor_tensor(t3[:], t3[:], t4[:], op=OP.subtract)
                    sub.append(t3)
                stage[c] = (pre, pd, sub, xp_r, xp_i)

            def stage_b(c, h):
                # R matmuls for one 512-col half: 5 feeds x {rre, rim}
                pre, pd, sub, xp_r, xp_i = stage[c]
                hs = slice(h * HS, (h + 1) * HS)
                p_rre = ppr.tile([128, HS], f32, tag="pr")
                p_rim = ppr.tile([128, HS], f32, tag="pr")
                for j, (wk, t) in enumerate(((0, pre[0]), (1, pre[1]), (1, pre[2]))):
                    wg = wg_sb[wk]
                    st = (j == 0)
                    nc.tensor.matmul(p_rre[:NO], wg[:128, 0:NO], t[:, hs], start=st, stop=False)
                    nc.tensor.matmul(p_rim[:NO], wg[:128, NO:2 * NO], t[:, hs], start=st, stop=False)
                wd = fw["wd"]
                nc.tensor.matmul(p_rre[:NO], wd[:, 0:NO], pd[:, hs], start=False, stop=False)
                nc.tensor.matmul(p_rim[:NO], wd[:, NO:2 * NO], pd[:, hs], start=False, stop=False)
                for k, (r0, rk) in enumerate(KSP_IM):
                    wi = wi_sb[k]
                    sp = (k == 1)
                    nc.tensor.matmul(p_rre[:NO], wi[:rk, 0:NO], sub[k][:, hs], start=False, stop=sp)
                    nc.tensor.matmul(p_rim[:NO], wi[:rk, NO:2 * NO], sub[k][:, hs], start=False, stop=sp)
                rstage[(c, h)] = (p_rre, p_rim)

            def copy_r(c, h):
                # PSUM -> SBUF bf16 into the full-chunk R tiles
                if h == 0:
                    rre = rp.tile([NO, NS], bf16, tag="rres")
                    rim = rp.tile([NO, NS], bf16, tag="rims")
                    rstage[c] = (rre, rim)
                rre, rim = rstage[c]
                p_rre, p_rim = rstage.pop((c, h))
                hs = slice(h * HS, (h + 1) * HS)
                nc.scalar.copy(rre[:, hs], p_rre[:NO])
                nc.scalar.copy(rim[:, hs], p_rim[:NO])

            def stage_u(c):
                # T products U = XP * R on the full 1024-col chunk (DVE)
                pre, pd, sub, xp_r, xp_i = stage.pop(c)
                rre, rim = rstage.pop(c)
                qs = slice(0, NS)
                lstage.pop(c, None)
                u1 = tp.tile([NO, NS], bf16, tag="u1")
                u2 = tp.tile([NO, NS], bf16, tag="u2")
                u3 = tp.tile([NO, NS], bf16, tag="u3")
                u4 = tp.tile([NO, NS], bf16, tag="u4")
                nc.vector.tensor_tensor(u1[:], xp_r[:, qs], rre[:], op=OP.mult)
                nc.vector.tensor_tensor(u2[:], xp_i[:, qs], rim[:], op=OP.mult)
                nc.vector.tensor_tensor(u3[:], xp_r[:, qs], rim[:], op=OP.mult)
                nc.vector.tensor_tensor(u4[:], xp_i[:, qs], rre[:], op=OP.mult)
                ustage[c] = (u1, u2, u3, u4)

            def stage_fin(c):
                # fin contraction for both halves into one full [40, HS] PSUM
                # tile: half A lands in rows 0:8, half B in rows 32:40, zeros
                # elsewhere (the zero weight columns write them explicitly)
                us = ustage.pop(c)
                fin_sb = fw["fin"]
                p_h1 = pp1.tile([40, HS], f32, tag="ph1")
                for h in (0, 1):
                    hs = slice(h * HS, (h + 1) * HS)
                    for j in range(4):
                        c0 = j * 80 + h * 40
                        nc.tensor.matmul(p_h1[0:40], fin_sb[:, c0:c0 + 40], us[j][:, hs],
                                         start=(h == 0 and j == 0), stop=(h == 1 and j == 3))
                h1s = tp.tile([40, HS], bf16, tag="h1s")
                lrelu(h1s[:], p_h1[0:40])
                tstage[c] = h1s

            def stage_tail(c):
                # paired MLP tail: w2 -> lrelu -> w3 -> E copy -> OUT accum
                h1s = tstage.pop(c)
                p_h2 = pp2.tile([80, HS], f32, tag="ph2")
                nc.tensor.matmul(p_h2[:80], fw["w2"][:], h1s[:], start=True, stop=True)
                h2s = tp.tile([80, HS], bf16, tag="h2s")
                lrelu(h2s[:], p_h2[:80])
                p_e = ppe.tile([8, HS], f32, tag="pe")
                nc.tensor.matmul(p_e[:8], fw["w3"][:], h2s[:], start=True, stop=True)
                eab = tp.tile([8, HS], f32, tag="eab")
                nc.scalar.copy(eab[:], p_e[:8])
                cs = slice(c * HS, (c + 1) * HS)
                nc.gpsimd.dma_start(out=outD[:, cs], in_=eab[:], accum_op=OP.add)

            # ---------------- schedule ----------------
            # consts first (gpsimd SWDGE, absorbs the one-time Q7 IRAM
            # load); block 0 leads the HWDGE queues so the first products
            # start as early as possible
            emit_consts()
            load_block(0)
            stage_a(0)
            # pre-fill OUT with the center taps; E accumulates onto it
            nc.gpsimd.dma_start(out=outD[:, :], in_=ctrD[:, :])
            load_block(1)
            for c in range(nchunk):
                if c >= 2:
                    stage_u(c - 2)
                if c + 2 < nchunk:
                    load_block(c + 2)
                # R matmuls for chunk c BEFORE emitting products(c+1): keeps
                # PE's semaphore waits scoped to already-finished DVE work
                stage_b(c, 0)
                copy_r(c, 0)
                stage_b(c, 1)
                copy_r(c, 1)
                if c + 1 < nchunk:
                    stage_a(c + 1)
                if c >= 2:
                    stage_fin(c - 2)
                if c >= 3:
                    stage_tail(c - 3)
            stage_u(nchunk - 2)
            stage_fin(nchunk - 2)
            stage_tail(nchunk - 3)
            stage_u(nchunk - 1)
            stage_fin(nchunk - 1)
            stage_tail(nchunk - 2)
            stage_tail(nchunk - 1)
    nc.compile()
    return nc


# ---------------------------------------------------------------------------
def _prep_core_inputs(inputs, static, folded):
    """Host-side gather + shard. Returns list of per-core in_maps."""
    xr = np.asarray(inputs["x_real"])     # [B, 41, 2]
    xi = np.asarray(inputs["x_imag"])
    t0 = np.ascontiguousarray(np.asarray(inputs["task_info"])[:, 0])
    # xq82 rows = q*41 + tap
    xrq = np.ascontiguousarray(xr.transpose(2, 1, 0).reshape(2 * MT, BATCH))
    xiq = np.ascontiguousarray(xi.transpose(2, 1, 0).reshape(2 * MT, BATCH))
    a_src, b_src, xp_src = static["a_src"], static["b_src"], static["xp_src"]
    xri = np.concatenate([xrq, xiq], axis=0)
    SAD2 = xri[static["d_src"]].astype(BF16)
    SAr = xrq[a_src].astype(BF16)
    SAi = xiq[a_src].astype(BF16)
    SBr = xrq[b_src].astype(BF16)
    SBi = xiq[b_src].astype(BF16)
    # P = 10^(t0/10) folded into the third-factor replicas (lrelu is
    # positively homogeneous; 1/NM stays folded in W3L)
    pex = (10.0 ** (t0[None, :] / 10.0)).astype(np.float32)
    XPr = (xrq[xp_src] * pex).astype(BF16)
    XPi = (xiq[xp_src] * pex).astype(BF16)
    # CTR2 [8, BATCH/2]: per chunk c, cols [c*HS,(c+1)*HS): rows 0:4 = half A
    # (samples c*NS..c*NS+HS), rows 4:8 = half B (c*NS+HS..(c+1)*NS)
    ctr4 = np.stack([xrq[LH], xrq[MT + LH], xiq[LH], xiq[MT + LH]], axis=0)
    ctr2 = ctr4.reshape(4, BATCH // NS, 2, HS).transpose(2, 0, 1, 3).reshape(8, BATCH // 2)
    shared = {"WGP": folded["WGP"].astype(BF16), "WIP": folded["WIP"].astype(BF16),
              "WD": folded["WD"].astype(BF16), "FINW": static["FINW"].astype(BF16),
              "W2L2": folded["W2L2"].astype(BF16), "W3L2": folded["W3L2"].astype(BF16)}
    def chunk_major(a, s):
        # [rows, BCORE] core slice -> [nchunk*rows, NS] chunk-major
        rows = a.shape[0]
        return np.ascontiguousarray(
            a[:, s].reshape(rows, NCHUNK, NS).transpose(1, 0, 2).reshape(NCHUNK * rows, NS))

    in_maps = []
    hc = BCORE // 2
    for c in range(NCORES):
        s = slice(c * BCORE, (c + 1) * BCORE)
        s2 = slice(c * hc, (c + 1) * hc)
        m = dict(shared)
        m["SAr"] = chunk_major(SAr, s)
        m["SAi"] = chunk_major(SAi, s)
        m["SBr"] = chunk_major(SBr, s)
        m["SBi"] = chunk_major(SBi, s)
        m["SAD2"] = chunk_major(SAD2, s)
        m["XPr"] = chunk_major(XPr, s)
        m["XPi"] = chunk_major(XPi, s)
        m["CTR2"] = np.ascontiguousarray(ctr2[:, s2])
        in_maps.append(m)
    return in_maps


def unshuffle_out2(o8, bcore=BCORE):
    """OUT2 [8, bcore/2] -> [bcore, 2, 2] (sample, pol, re/im)."""
    nch = bcore // NS
    o = o8.reshape(2, 4, nch, HS)          # (half, comp, chunk, col)
    out = np.empty((bcore, NM, 2), np.float32)
    flat = o.transpose(2, 0, 3, 1).reshape(bcore, 4)   # (chunk, half, col, comp)
    out[:, 0, 0] = flat[:, 0]
    out[:, 1, 0] = flat[:, 1]
    out[:, 0, 1] = flat[:, 2]
    out[:, 1, 1] = flat[:, 3]
    return out


_CACHE = {}


def kernel(**inputs):
    from concourse.bass_utils import run_bass_kernel_spmd

    static = build_static()
    folded = fold_weights(
        np.asarray(inputs["W1_real"]), np.asarray(inputs["W1_imag"]),
        np.asarray(inputs["W2_real"]), np.asarray(inputs["W2_imag"]),
        np.asarray(inputs["W3_real"]), np.asarray(inputs["W3_imag"]),
    )
    if "nc" not in _CACHE:
        _CACHE["nc"] = build_nc()
    nc = _CACHE["nc"]
    in_maps = _prep_core_inputs(inputs, static, folded)
    res = run_bass_kernel_spmd(nc, in_maps, list(range(NCORES)))
    out = np.empty((BATCH, NM, 2), np.float32)
    for c in range(NCORES):
        o8 = res.results[c]["OUT2"]
        s = slice(c * BCORE, (c + 1) * BCORE)
        out[s] = unshuffle_out2(o8)
    return out


# revision 54
# speedup vs baseline: 1.1091x; 1.0913x over previous
"""EqPBCNN (perturbation-based nonlinearity compensation NN) Trainium2 Bass kernel.

Data-parallel over 8 NeuronCores: batch 65536 -> 8192 per core.

Math (per sample):
  G_(a,b) = sum_q x[a,q] * conj(x[b,q])      (pol-independent; pairs (a,b)=(n+L, m+n+L))
  h1[p,o] = sum_m x[m,p] * R[p,o,m],  R = sum_n W1'[p,o,(m,n)] * G
  h2 = CLrelu(h1) @ W2^T; E = CLrelu(h2) @ W3^T
  out = x[center,p] + E * 10^(task0/10)/2

v3 design (engine-balanced rework of v2, ~132us -> ~115-120us):
  - conjugate-canonical pairs (148 of 175) folded into R weights (as v2);
    host pre-gathers pair stacks SAr/SAi/SBr/SBi [296, B/8] bf16 + SAD2/XP,
    stored chunk-major in DRAM ([nchunk*rows, 1024]).
  - P = 10^(task0/10) folded into XP on host (lrelu is positively
    homogeneous, so scaling the third factor x_m scales E linearly) -- the
    per-sample E*P multiply disappears.
  - DVE per chunk (1024 cols): 9 product mults + 1 merge add + 2 subtracts
    + 4 U mults, all bf16 SBUF (2x_1P mode, ~600ns each). Keeping ACT and
    GPSIMD quiet matters: concurrent ACT/GPSIMD SBUF traffic inflates every
    DVE op by ~20% (shared-port contention), so GPSIMD only does the OUT
    SWDGE and ACT only the R copies + lrelus.
  - PE per half (512 cols): 11 R matmuls (Pre0, P1k1, P2k1 w/ shared wg1,
    diag, Gim k0/k1 for re+im); per chunk: 8 fin (full [0:40] PSUM writes
    via zero-padded FINW blocks) + 1 w2 + 1 w3 (tail paired across the two
    halves via block-diagonal W2/W3).
  - 3-deep software pipeline: products(c+1) | R-matmuls(c) | U(c-2),
    fin(c-2), tail(c-3); loads 2 blocks ahead.
  - OUT layout [8, B/8/2]: rows 0:4 = half A (re0,re1,im0,im1), 4:8 half B;
    prefilled with center taps, E accumulated via SWDGE.
"""
import numpy as np
import ml_dtypes

BF16 = ml_dtypes.bfloat16

# ---------------- problem constants (hardcoded; must match reference) -------
BATCH = 65536
MT, LH = 41, 20          # filter taps, half window
NM = 2                   # modes / polarizations
H1, H2 = 2, 10
SLOPE = 0.01
NCORES = 8
BCORE = BATCH // NCORES  # 8192
NS = 1024                # columns per compute chunk
NCHUNK = BCORE // NS     # 8
LS = 1024                # columns per DMA load block (1 chunk)
HS = NS // 2             # 512-col half for PSUM/matmul stages

# ---------------- triplet / canonical-pair tables ---------------------------
_idx = [(m, n) for m in range(-LH, LH + 1) for n in range(-LH, LH + 1)
        if abs(m * n) <= LH and abs(m + n) <= LH and n >= m]
H = len(_idx)            # 175
SYM = np.where(np.array([m for m, n in _idx]) != np.array([n for m, n in _idx]),
               2.0, 1.0).astype(np.float32)
M_VALS = sorted(set(m for m, n in _idx))     # 25 distinct m values
NMV = len(M_VALS)
M_POS = {m: i for i, m in enumerate(M_VALS)}
NO = H1 * NMV * NM       # 100 rows of R/T space: (o, mi, p)

# canonical pairs: key (a,b) a<=b; triplet h -> (pair index, Gim sign)
_ckeys = {}
_tripmap = []
for (m, n) in _idx:
    a, b = n + LH, m + n + LH
    key, s = ((a, b), 1.0) if a <= b else ((b, a), -1.0)
    _ckeys[key] = None
    _tripmap.append((key, s))
POFF = sorted([k for k in _ckeys if k[0] < k[1]], key=lambda k: (k[1] - k[0], k[0]))
PDIAG = sorted([k for k in _ckeys if k[0] == k[1]])
NOFF, NDIAG = len(POFF), len(PDIAG)          # 127, 21
NPAIR = NOFF + NDIAG                          # 148
# stack rows: q0-off(127), q1-off(127), q0-diag(21), q1-diag(21)
NROWS_RE = 2 * NPAIR                          # 296 (P1/P2/Gre rows)
NROWS_IM = 2 * NOFF                           # 254 (P3/P4/Gim rows)
_pairpos = {}
for i, k in enumerate(POFF):
    _pairpos[k] = ('off', i)
for i, k in enumerate(PDIAG):
    _pairpos[k] = ('diag', i)


def _stack_row(kind, i, q):
    return q * NOFF + i if kind == 'off' else NROWS_IM + q * NDIAG + i


def _orow(o, mi, p):
    return (o * NMV + mi) * NM + p


def _hrow(p, o, comp):
    return (p * H1 + o) * 2 + comp


def _h2row(p, q, comp):
    return (p * H2 + q) * 2 + comp


# split boundaries of the 296-row stacks
KSP_RE = [(0, 128), (128, 128), (256, NROWS_RE - 256)]   # 128,128,40
KSP_IM = [(0, 128), (128, NROWS_IM - 128)]               # 128,126


def build_static():
    """Gather row tables (host side) + final/contraction constants."""
    # source rows within xq82 [2*41, BCORE] (rows = q*41 + tap)
    a_src = np.zeros(NROWS_RE, np.int64)
    b_src = np.zeros(NROWS_RE, np.int64)
    for key in POFF + PDIAG:
        kind, i = _pairpos[key]
        a, b = key
        for q in range(NM):
            r = _stack_row(kind, i, q)
            a_src[r] = q * MT + a
            b_src[r] = q * MT + b
    # xrep rows (o, mi, p) -> tap m+L, pol p
    xp_src = np.zeros(NO, np.int64)
    for o in range(H1):
        for mi, mv in enumerate(M_VALS):
            for p in range(NM):
                xp_src[_orow(o, mi, p)] = p * MT + (mv + LH)
    # final contraction: per u-group j (U1 +re, U2 -re, U3 +im, U4 +im), a
    # [100, 80] lhsT pair: cols 0:40 = half-A block (h1 rows 0:8, rest 0),
    # cols 40:80 = half-B block (h1 rows 32:40, rest 0). Zero columns make
    # every fin matmul write the full [0:40] PSUM range (no garbage rows).
    fin1 = np.zeros((NO, 4, 8), np.float32)
    for o in range(H1):
        for mi in range(NMV):
            for p in range(NM):
                r = _orow(o, mi, p)
                fin1[r, 0, _hrow(p, o, 0)] = 1.0
                fin1[r, 1, _hrow(p, o, 0)] = -1.0
                fin1[r, 2, _hrow(p, o, 1)] = 1.0
                fin1[r, 3, _hrow(p, o, 1)] = 1.0
    FINW = np.zeros((NO, 4, 2, 40), np.float32)
    FINW[:, :, 0, 0:8] = fin1
    FINW[:, :, 1, 32:40] = fin1
    FINW = FINW.reshape(NO, 320)
    d_src = np.concatenate([a_src[256:NROWS_RE], MT * NM + a_src[256:NROWS_RE]])
    return {"a_src": a_src, "b_src": b_src, "xp_src": xp_src, "d_src": d_src,
            "FINW": FINW}


def fold_weights(W1r, W1i, W2r, W2i, W3r, W3i):
    """Fold W1 (with SYM, pol-sum dup, conj-pair signs) into R-matmul lhsT."""
    Wr = W1r * SYM[None, None, :]   # [p, o, h]
    Wi = W1i * SYM[None, None, :]
    # WG [296, 200]: cols 0:100 -> Rre (+Wr), 100:200 -> Rim (+Wi); fed by Pre=P1+P2
    WG = np.zeros((NROWS_RE, 2 * NO), np.float32)
    # WI [254, 200]: Gim rows; cols 0:100 -> Rre (-s*Wi), 100:200 -> Rim (+s*Wr)
    WI = np.zeros((NROWS_IM, 2 * NO), np.float32)
    for h, (mn, (key, s)) in enumerate(zip(_idx, _tripmap)):
        m, n = mn
        kind, i = _pairpos[key]
        mi = M_POS[m]
        for p in range(NM):
            for o in range(H1):
                c = _orow(o, mi, p)
                for q in range(NM):
                    r = _stack_row(kind, i, q)
                    WG[r, c] += Wr[p, o, h]
                    WG[r, NO + c] += Wi[p, o, h]
                    if kind == 'off':
                        WI[r, c] += -s * Wi[p, o, h]
                        WI[r, NO + c] += s * Wr[p, o, h]
    WGP = np.zeros((3, 128, 2 * NO), np.float32)
    for k, (r0, rk) in enumerate(KSP_RE):
        WGP[k, :rk, :] = WG[r0:r0 + rk, :]
    WIP = np.zeros((2, 128, 2 * NO), np.float32)
    for k, (r0, rk) in enumerate(KSP_IM):
        WIP[k, :rk, :] = WI[r0:r0 + rk, :]
    WD = np.concatenate([WG[256:NROWS_RE, :], WG[256:NROWS_RE, :]], axis=0)
    # W2 lhsT [8, 40] on h1 rows (p,o,comp)
    W2L = np.zeros((8, 2 * H2 * NM), np.float32)
    for p in range(NM):
        for q in range(H2):
            for o in range(H1):
                W2L[_hrow(p, o, 0), _h2row(p, q, 0)] += W2r[p, q, o]
                W2L[_hrow(p, o, 1), _h2row(p, q, 0)] -= W2i[p, q, o]
                W2L[_hrow(p, o, 0), _h2row(p, q, 1)] += W2i[p, q, o]
                W2L[_hrow(p, o, 1), _h2row(p, q, 1)] += W2r[p, q, o]
    # W3 lhsT [40, 4]: out rows [re_p0, re_p1, im_p0, im_p1]; 1/NM folded
    W3L = np.zeros((2 * H2 * NM, 4), np.float32)
    s3 = 1.0 / NM
    for p in range(NM):
        for q in range(H2):
            W3L[_h2row(p, q, 0), 0 + p] += W3r[p, 0, q] * s3
            W3L[_h2row(p, q, 1), 0 + p] -= W3i[p, 0, q] * s3
            W3L[_h2row(p, q, 0), 2 + p] += W3i[p, 0, q] * s3
            W3L[_h2row(p, q, 1), 2 + p] += W3r[p, 0, q] * s3
    # paired-tail block-diagonal versions: half A -> h1 rows 0:8, E rows 0:4;
    # half B -> h1 rows 32:40 (PSUM col-tile position constraint), E rows 4:8
    W2L2 = np.zeros((40, 80), np.float32)
    W2L2[0:8, 0:40] = W2L
    W2L2[32:40, 40:80] = W2L
    W3L2 = np.zeros((80, 8), np.float32)
    W3L2[0:40, 0:4] = W3L
    W3L2[40:80, 4:8] = W3L
    return {"WGP": WGP, "WIP": WIP, "WD": WD, "W2L2": W2L2, "W3L2": W3L2}


# ---------------------------------------------------------------------------
def build_nc(bcore=BCORE, lrelu_mode="act"):
    """Build the Bass program for one core processing `bcore` samples."""
    import concourse.bass as bass
    import concourse.bacc as bacc
    import concourse.mybir as mybir
    from concourse.tile import TileContext
    import bass_rust

    nchunk = bcore // NS
    assert nchunk * NS == bcore and nchunk % 2 == 0
    f32 = mybir.dt.float32
    bf16 = mybir.dt.bfloat16
    AF = bass_rust.ActivationFunctionType
    OP = mybir.AluOpType

    nc = bacc.Bacc(None, target_bir_lowering=False, debug=False)
    # stack tensors are chunk-major: [nchunk * rows, NS] so each chunk's
    # block is one contiguous DRAM region (64KB DMA descriptors, not 2KB)
    saR = nc.declare_dram_parameter("SAr", [nchunk * NROWS_RE, NS], bf16, isOutput=False)
    saI = nc.declare_dram_parameter("SAi", [nchunk * NROWS_RE, NS], bf16, isOutput=False)
    sbR = nc.declare_dram_parameter("SBr", [nchunk * NROWS_RE, NS], bf16, isOutput=False)
    sbI = nc.declare_dram_parameter("SBi", [nchunk * NROWS_RE, NS], bf16, isOutput=False)
    sadD = nc.declare_dram_parameter("SAD2", [nchunk * 80, NS], bf16, isOutput=False)
    xpR = nc.declare_dram_parameter("XPr", [nchunk * NO, NS], bf16, isOutput=False)
    xpI = nc.declare_dram_parameter("XPi", [nchunk * NO, NS], bf16, isOutput=False)
    ctrD = nc.declare_dram_parameter("CTR2", [8, bcore // 2], f32, isOutput=False)
    wgD = nc.declare_dram_parameter("WGP", [3, 128, 2 * NO], bf16, isOutput=False)
    wiD = nc.declare_dram_parameter("WIP", [2, 128, 2 * NO], bf16, isOutput=False)
    wdD = nc.declare_dram_parameter("WD", [80, 2 * NO], bf16, isOutput=False)
    finD = nc.declare_dram_parameter("FINW", [NO, 320], bf16, isOutput=False)
    w2D = nc.declare_dram_parameter("W2L2", [40, 80], bf16, isOutput=False)
    w3D = nc.declare_dram_parameter("W3L2", [80, 8], bf16, isOutput=False)
    outD = nc.declare_dram_parameter("OUT2", [8, bcore // 2], f32, isOutput=True)

    with TileContext(nc) as tc:
        with (
            tc.tile_pool(name="consts", bufs=1) as cp,
            tc.tile_pool(name="mega", bufs=3) as mp,
            tc.tile_pool(name="xpp", bufs=5) as xpp,
            tc.tile_pool(name="prod", bufs=3) as up,
            tc.tile_pool(name="rcopy", bufs=3) as rp,
            tc.tile_pool(name="tt", bufs=3) as tp,
            tc.tile_pool(name="psumr", bufs=4, space="PSUM") as ppr,
            tc.tile_pool(name="psum1", bufs=2, space="PSUM") as pp1,
            tc.tile_pool(name="psum2", bufs=1, space="PSUM") as pp2,
            tc.tile_pool(name="psume", bufs=1, space="PSUM") as ppe,
        ):
            def const_tile(src_ap, name):
                # consts go through the gpsimd SWDGE queue: its completion
                # semaphore is separate from the HWDGE load queues, so the
                # first R matmul doesn't wait behind block loads
                tr = cp.tile(list(src_ap.shape), bf16, name=name)
                nc.gpsimd.dma_start(out=tr[:], in_=src_ap)
                return tr

            wg_sb, wi_sb = [None] * 3, [None] * 2
            fw = {}

            def emit_consts():
                for k in range(3):
                    wg_sb[k] = const_tile(wgD[k], f"wg{k}")
                for k in range(2):
                    wi_sb[k] = const_tile(wiD[k], f"wi{k}")
                fw["wd"] = const_tile(wdD[:], "wd")
                fw["fin"] = const_tile(finD[:], "fin")
                fw["w2"] = const_tile(w2D[:], "w2")
                fw["w3"] = const_tile(w3D[:], "w3")

            stage = {}           # c -> product tiles etc for chunk c
            lstage = {}          # lb -> loaded stack tiles (2048-col)
            rstage = {}          # c -> (rre_s, rim_s) [100, 1024] bf16
            ustage = {}          # c -> u tiles
            tstage = {}          # c -> tail intermediates

            def lrelu(dst, src_ap):
                if lrelu_mode == "act":
                    nc.scalar.activation(dst, src_ap, AF.Lrelu, alpha=SLOPE)
                else:
                    nc.vector.tensor_scalar_mul(dst, src_ap, SLOPE)
                    nc.vector.tensor_tensor(dst, dst, src_ap, op=OP.max)

            def load_block(lb):
                sa_r, sa_i, sb_r, sb_i = [], [], [], []
                for k, (r0, rk) in enumerate(KSP_RE[:2]):
                    for nm_, src_, lst, eng in (
                        (f"sar{k}", saR, sa_r, nc.sync), (f"sai{k}", saI, sa_i, nc.sync),
                        (f"sbr{k}", sbR, sb_r, nc.scalar), (f"sbi{k}", sbI, sb_i, nc.scalar)):
                        t = mp.tile([rk, LS], bf16, tag=nm_)
                        b0 = lb * NROWS_RE + r0
                        eng.dma_start(out=t[:], in_=src_[b0:b0 + rk, :])
                        lst.append(t)
                sad = mp.tile([80, LS], bf16, tag="sad")
                nc.sync.dma_start(out=sad[:], in_=sadD[lb * 80:(lb + 1) * 80, :])
                xp_r = xpp.tile([NO, LS], bf16, tag="xpr")
                xp_i = xpp.tile([NO, LS], bf16, tag="xpi")
                nc.sync.dma_start(out=xp_r[:], in_=xpR[lb * NO:(lb + 1) * NO, :])
                nc.scalar.dma_start(out=xp_i[:], in_=xpI[lb * NO:(lb + 1) * NO, :])
                lstage[lb] = (sa_r, sa_i, sb_r, sb_i, sad, xp_r, xp_i)

            def stage_a(c):
                # products for chunk c from load block c
                sa_r, sa_i, sb_r, sb_i, sad, xp_r, xp_i = lstage[c]
                qs = slice(0, NS)
                pre, sub = [], []
                # P1, P2 products; k=0 pair merged on DVE, k=1 pair left
                # separate (PE accumulates both with the same wg1 weights)
                for k in range(2):
                    t1 = up.tile([128, NS], bf16, tag=f"p1_{k}")
                    t2 = up.tile([128, NS], bf16, tag=f"p2_{k}")
                    nc.vector.tensor_tensor(t1[:], sa_r[k][:, qs], sb_r[k][:, qs], op=OP.mult)
                    nc.vector.tensor_tensor(t2[:], sa_i[k][:, qs], sb_i[k][:, qs], op=OP.mult)
                    if k == 0:
                        nc.vector.tensor_tensor(t1[:], t1[:], t2[:], op=OP.add)
                        pre.append(t1)
                    else:
                        pre.append(t1)
                        pre.append(t2)
                pd = up.tile([80, NS], bf16, tag="pd")
                nc.vector.tensor_tensor(pd[:], sad[:, qs], sad[:, qs], op=OP.mult)
                # P3 - P4 for k=0,1 (Gim feed)
                for k, (r0, rk) in enumerate(KSP_IM):
                    t3 = up.tile([rk, NS], bf16, tag=f"p3_{k}")
                    t4 = up.tile([rk, NS], bf16, tag=f"p4_{k}")
                    nc.vector.tensor_tensor(t3[:], sa_i[k][:rk, qs], sb_r[k][:rk, qs], op=OP.mult)
                    nc.vector.tensor_tensor(t4[:], sa_r[k][:rk, qs], sb_i[k][:rk, qs], op=OP.mult)
                    nc.vector.tensor_tensor(t3[:], t3[:], t4[:], op=OP.subtract)
                    sub.append(t3)
                stage[c] = (pre, pd, sub, xp_r, xp_i)

            def stage_b(c, h):
                # R matmuls for one 512-col half: 5 feeds x {rre, rim}
                pre, pd, sub, xp_r, xp_i = stage[c]
                hs = slice(h * HS, (h + 1) * HS)
                p_rre = ppr.tile([128, HS], f32, tag="pr")
                p_rim = ppr.tile([128, HS], f32, tag="pr")
                for j, (wk, t) in enumerate(((0, pre[0]), (1, pre[1]), (1, pre[2]))):
                    wg = wg_sb[wk]
                    st = (j == 0)
                    nc.tensor.matmul(p_rre[:NO], wg[:128, 0:NO], t[:, hs], start=st, stop=False)
                    nc.tensor.matmul(p_rim[:NO], wg[:128, NO:2 * NO], t[:, hs], start=st, stop=False)
                wd = fw["wd"]
                nc.tensor.matmul(p_rre[:NO], wd[:, 0:NO], pd[:, hs], start=False, stop=False)
                nc.tensor.matmul(p_rim[:NO], wd[:, NO:2 * NO], pd[:, hs], start=False, stop=False)
                for k, (r0, rk) in enumerate(KSP_IM):
                    wi = wi_sb[k]
                    sp = (k == 1)
                    nc.tensor.matmul(p_rre[:NO], wi[:rk, 0:NO], sub[k][:, hs], start=False, stop=sp)
                    nc.tensor.matmul(p_rim[:NO], wi[:rk, NO:2 * NO], sub[k][:, hs], start=False, stop=sp)
                rstage[(c, h)] = (p_rre, p_rim)

            def copy_r(c, h):
                # PSUM -> SBUF bf16 into the full-chunk R tiles
                if h == 0:
                    rre = rp.tile([NO, NS], bf16, tag="rres")
                    rim = rp.tile([NO, NS], bf16, tag="rims")
                    rstage[c] = (rre, rim)
                rre, rim = rstage[c]
                p_rre, p_rim = rstage.pop((c, h))
                hs = slice(h * HS, (h + 1) * HS)
                nc.scalar.copy(rre[:, hs], p_rre[:NO])
                nc.scalar.copy(rim[:, hs], p_rim[:NO])

            def stage_u(c):
                # T products U = XP * R on the full 1024-col chunk (DVE)
                pre, pd, sub, xp_r, xp_i = stage.pop(c)
                rre, rim = rstage.pop(c)
                qs = slice(0, NS)
                lstage.pop(c, None)
                u1 = tp.tile([NO, NS], bf16, tag="u1")
                u2 = tp.tile([NO, NS], bf16, tag="u2")
                u3 = tp.tile([NO, NS], bf16, tag="u3")
                u4 = tp.tile([NO, NS], bf16, tag="u4")
                nc.vector.tensor_tensor(u1[:], xp_r[:, qs], rre[:], op=OP.mult)
                nc.vector.tensor_tensor(u2[:], xp_i[:, qs], rim[:], op=OP.mult)
                nc.vector.tensor_tensor(u3[:], xp_r[:, qs], rim[:], op=OP.mult)
                nc.vector.tensor_tensor(u4[:], xp_i[:, qs], rre[:], op=OP.mult)
                ustage[c] = (u1, u2, u3, u4)

            def stage_fin(c):
                # fin contraction for both halves into one full [40, HS] PSUM
                # tile: half A lands in rows 0:8, half B in rows 32:40, zeros
                # elsewhere (the zero weight columns write them explicitly)
                us = ustage.pop(c)
                fin_sb = fw["fin"]
                p_h1 = pp1.tile([40, HS], f32, tag="ph1")
                for h in (0, 1):
                    hs = slice(h * HS, (h + 1) * HS)
                    for j in range(4):
                        c0 = j * 80 + h * 40
                        nc.tensor.matmul(p_h1[0:40], fin_sb[:, c0:c0 + 40], us[j][:, hs],
                                         start=(h == 0 and j == 0), stop=(h == 1 and j == 3))
                h1s = tp.tile([40, HS], bf16, tag="h1s")
                lrelu(h1s[:], p_h1[0:40])
                tstage[c] = h1s

            def stage_tail(c):
                # paired MLP tail: w2 -> lrelu -> w3 -> E copy -> OUT accum
                h1s = tstage.pop(c)
                p_h2 = pp2.tile([80, HS], f32, tag="ph2")
                nc.tensor.matmul(p_h2[:80], fw["w2"][:], h1s[:], start=True, stop=True)
                h2s = tp.tile([80, HS], bf16, tag="h2s")
                lrelu(h2s[:], p_h2[:80])
                p_e = ppe.tile([8, HS], f32, tag="pe")
                nc.tensor.matmul(p_e[:8], fw["w3"][:], h2s[:], start=True, stop=True)
                eab = tp.tile([8, HS], f32, tag="eab")
                nc.scalar.copy(eab[:], p_e[:8])
                cs = slice(c * HS, (c + 1) * HS)
                nc.gpsimd.dma_start(out=outD[:, cs], in_=eab[:], accum_op=OP.add)

            # ---------------- schedule ----------------
            # consts first (gpsimd SWDGE, absorbs the one-time Q7 IRAM
            # load); block 0 leads the HWDGE queues so the first products
            # start as early as possible
            emit_consts()
            load_block(0)
            stage_a(0)
            # pre-fill OUT with the center taps; E accumulates onto it
            nc.gpsimd.dma_start(out=outD[:, :], in_=ctrD[:, :])
            load_block(1)
            for c in range(nchunk):
                if c >= 2:
                    stage_u(c - 2)
                if c + 2 < nchunk:
                    load_block(c + 2)
                # R matmuls for chunk c BEFORE emitting products(c+1): keeps
                # PE's semaphore waits scoped to already-finished DVE work
                stage_b(c, 0)
                copy_r(c, 0)
                stage_b(c, 1)
                copy_r(c, 1)
                if c + 1 < nchunk:
                    stage_a(c + 1)
                if c >= 2:
                    stage_fin(c - 2)
                if c >= 3:
                    stage_tail(c - 3)
            stage_u(nchunk - 2)
            stage_fin(nchunk - 2)
            stage_tail(nchunk - 3)
            stage_u(nchunk - 1)
            stage_fin(nchunk - 1)
            stage_tail(nchunk - 2)
            stage_tail(nchunk - 1)
    nc.compile()
    return nc


# ---------------------------------------------------------------------------
def _prep_core_inputs(inputs, static, folded):
    """Host-side gather + shard. Returns list of per-core in_maps."""
    xr = np.asarray(inputs["x_real"])     # [B, 41, 2]
    xi = np.asarray(inputs["x_imag"])
    t0 = np.ascontiguousarray(np.asarray(inputs["task_info"])[:, 0])
    # xq82 rows = q*41 + tap
    xrq = np.ascontiguousarray(xr.transpose(2, 1, 0).reshape(2 * MT, BATCH))
    xiq = np.ascontiguousarray(xi.transpose(2, 1, 0).reshape(2 * MT, BATCH))
    a_src, b_src, xp_src = static["a_src"], static["b_src"], static["xp_src"]
    xri = np.concatenate([xrq, xiq], axis=0)
    SAD2 = xri[static["d_src"]].astype(BF16)
    SAr = xrq[a_src].astype(BF16)
    SAi = xiq[a_src].astype(BF16)
    SBr = xrq[b_src].astype(BF16)
    SBi = xiq[b_src].astype(BF16)
    # P = 10^(t0/10) folded into the third-factor replicas (lrelu is
    # positively homogeneous; 1/NM stays folded in W3L)
    pex = (10.0 ** (t0[None, :] / 10.0)).astype(np.float32)
    XPr = (xrq[xp_src] * pex).astype(BF16)
    XPi = (xiq[xp_src] * pex).astype(BF16)
    # CTR2 [8, BATCH/2]: per chunk c, cols [c*HS,(c+1)*HS): rows 0:4 = half A
    # (samples c*NS..c*NS+HS), rows 4:8 = half B (c*NS+HS..(c+1)*NS)
    ctr4 = np.stack([xrq[LH], xrq[MT + LH], xiq[LH], xiq[MT + LH]], axis=0)
    ctr2 = ctr4.reshape(4, BATCH // NS, 2, HS).transpose(2, 0, 1, 3).reshape(8, BATCH // 2)
    shared = {"WGP": folded["WGP"].astype(BF16), "WIP": folded["WIP"].astype(BF16),
              "WD": folded["WD"].astype(BF16), "FINW": static["FINW"].astype(BF16),
              "W2L2": folded["W2L2"].astype(BF16), "W3L2": folded["W3L2"].astype(BF16)}
    def chunk_major(a, s):
        # [rows, BCORE] core slice -> [nchunk*rows, NS] chunk-major
        rows = a.shape[0]
        return np.ascontiguousarray(
            a[:, s].reshape(rows, NCHUNK, NS).transpose(1, 0, 2).reshape(NCHUNK * rows, NS))

    in_maps = []
    hc = BCORE // 2
    for c in range(NCORES):
        s = slice(c * BCORE, (c + 1) * BCORE)
        s2 = slice(c * hc, (c + 1) * hc)
        m = dict(shared)
        m["SAr"] = chunk_major(SAr, s)
        m["SAi"] = chunk_major(SAi, s)
        m["SBr"] = chunk_major(SBr, s)
        m["SBi"] = chunk_major(SBi, s)
        m["SAD2"] = chunk_major(SAD2, s)
        m["XPr"] = chunk_major(XPr, s)
        m["XPi"] = chunk_major(XPi, s)
        m["CTR2"] = np.ascontiguousarray(ctr2[:, s2])
        in_maps.append(m)
    return in_maps


def unshuffle_out2(o8, bcore=BCORE):
    """OUT2 [8, bcore/2] -> [bcore, 2, 2] (sample, pol, re/im)."""
    nch = bcore // NS
    o = o8.reshape(2, 4, nch, HS)          # (half, comp, chunk, col)
    out = np.empty((bcore, NM, 2), np.float32)
    flat = o.transpose(2, 0, 3, 1).reshape(bcore, 4)   # (chunk, half, col, comp)
    out[:, 0, 0] = flat[:, 0]
    out[:, 1, 0] = flat[:, 1]
    out[:, 0, 1] = flat[:, 2]
    out[:, 1, 1] = flat[:, 3]
    return out


_CACHE = {}


def kernel(**inputs):
    from concourse.bass_utils import run_bass_kernel_spmd

    static = build_static()
    folded = fold_weights(
        np.asarray(inputs["W1_real"]), np.asarray(inputs["W1_imag"]),
        np.asarray(inputs["W2_real"]), np.asarray(inputs["W2_imag"]),
        np.asarray(inputs["W3_real"]), np.asarray(inputs["W3_imag"]),
    )
    if "nc" not in _CACHE:
        _CACHE["nc"] = build_nc()
    nc = _CACHE["nc"]
    in_maps = _prep_core_inputs(inputs, static, folded)
    res = run_bass_kernel_spmd(nc, in_maps, list(range(NCORES)))
    out = np.empty((BATCH, NM, 2), np.float32)
    for c in range(NCORES):
        o8 = res.results[c]["OUT2"]
        s = slice(c * BCORE, (c + 1) * BCORE)
        out[s] = unshuffle_out2(o8)
    return out
